# revision 11
# baseline (speedup 1.0000x reference)
"""Trainium2 Bass kernel for nn_AttentionWeightedRetrieval.

Math (reference):
    d_ij   = ||z_i - c_j||                         (N x C distances)
    c_pot_i = sum_j mu_j / (d_ij + eps)            (N,)
    q_pot   = same for the single query row
    logits  = -|q_pot - c_pot| / TEMP              (N,)  -- always <= 0
    attention = softmax(logits)                    (N,)
    query_output = query + dt * sum_j mu_j (c_j - q) / (r^3 + eps)   (1, D)

Device strategy (per core, candidates row-sharded 8 ways):
    - Host pre-transposes candidates into a "stacked" layout
      zstack[64*h + d, pair*512 + i] = z[(2*pair + h)*512 + i, d]
      so each matmul contracts over D on the partition axis with zero
      on-device transposes and fully contiguous DMA.
    - PE: d2acc = |z_i|^2 - 2 z.c_j via two K=64 matmuls into PSUM
      (lhsT = -2*centers^T for the z part, all-ones for the z^2 part;
      z^2 computed by one DVE multiply over the stacked tile).
    - ACT: dtil = sqrt(scale_j * d2acc + bias_j) = d_ij / mu_j
      with scale_j = 1/mu_j^2, bias_j = |c_j|^2/mu_j^2 (per-partition APs).
    - DVE: r = reciprocal_approx_fast(dtil) = mu_j / d_ij  (~51 ULP).
    - PE: pot_t = ones-indicator matmul accumulates sum_j r into row t of a
      single persistent PSUM bank -> after 124 subtiles the bank holds all
      63488 per-core potentials.
    - ACT: e = exp(-|pot - q_pot|/TEMP) (+ fused per-partition row sums).
    - Global softmax denominator: logits <= 0 by construction so no max
      subtraction is needed; per-core partial sums are combined at gather
      time (distributed-softmax all-gather-sum-exp, done on host).
"""

import numpy as np

N = 500000
C = 128
D = 64
EPS = 1e-6
TEMP = 0.1
DT = 0.1

NCORES = 8
SUB = 512            # candidates per matmul subtile
NSUB = 124           # subtiles per core
NPAIR = NSUB // 2
NPC = NSUB * SUB     # 63488 candidates per core
NPAD = NCORES * NPC  # 507904 padded total
GROUP = 4            # pairs per DMA / DVE batch

_NC = None           # compiled program cache (program is input-independent)


def _ensure_path():
    try:
        import concourse  # noqa: F401
    except ImportError:
        import sys
        for p in ("/opt/trn_rl_repo", "/root/.axon_site/_ro/trn_rl_repo"):
            if p not in sys.path:
                sys.path.insert(0, p)


def _build_program(nsub=NSUB, group=GROUP, bench_loops=None):
    """Build + compile the per-core Bass program (identical on all cores).

    bench_loops: if set, wrap the main tile loop in a hardware For_i that
    repeats it that many times (timing harness only — output is garbage
    except for the final iteration's pot, which is actually still correct
    since each iteration restarts the pot accumulation).
    """
    _ensure_path()
    from contextlib import ExitStack

    import concourse.bass as bass  # noqa: F401
    import concourse.tile as tile
    from concourse import bacc, mybir
    from concourse.dve_ops import RECIP_APPROX_FAST_CONSTS, RECIPROCAL_APPROX_FAST

    f32 = mybir.dt.float32
    f32r = mybir.dt.float32r  # fp32 bits, fast reduced-precision matmul path
    AF = mybir.ActivationFunctionType
    npair = nsub // 2

    nc = bacc.Bacc("TRN2", target_bir_lowering=False, debug=False)

    zstack_d = nc.dram_tensor("zstack", [128, npair * SUB], f32r, kind="ExternalInput")
    kxmz_d = nc.dram_tensor("kxmz", [128, 128], f32r, kind="ExternalInput")
    kxms_d = nc.dram_tensor("kxms", [128, 128], f32r, kind="ExternalInput")
    omat_d = nc.dram_tensor("omat", [128, 256], f32r, kind="ExternalInput")
    scale_d = nc.dram_tensor("scalev", [128, 1], f32, kind="ExternalInput")
    bias_d = nc.dram_tensor("biasv", [128, 1], f32, kind="ExternalInput")
    qneg_d = nc.dram_tensor("qneg", [128, 1], f32, kind="ExternalInput")
    eout_d = nc.dram_tensor("eout", [nsub, SUB], f32, kind="ExternalOutput")
    ssum_d = nc.dram_tensor("ssum", [128, 1], f32, kind="ExternalOutput")

    with tile.TileContext(nc) as tc, ExitStack() as ctx:
        consts = ctx.enter_context(tc.tile_pool(name="consts", bufs=1))
        zpool = ctx.enter_context(tc.tile_pool(name="zp", bufs=3))
        sqpool = ctx.enter_context(tc.tile_pool(name="sqp", bufs=3))
        dpool = ctx.enter_context(tc.tile_pool(name="dp", bufs=3))
        rpool = ctx.enter_context(tc.tile_pool(name="rp", bufs=3))
        pspool = ctx.enter_context(tc.tile_pool(name="ps", bufs=3, space="PSUM"))
        potpool = ctx.enter_context(tc.tile_pool(name="potp", bufs=1, space="PSUM"))

        kz = consts.tile([128, 128], f32r)
        nc.sync.dma_start(out=kz[:], in_=kxmz_d.ap())
        ks = consts.tile([128, 128], f32r)
        nc.sync.dma_start(out=ks[:], in_=kxms_d.ap())
        om = consts.tile([128, 256], f32r)
        nc.sync.dma_start(out=om[:], in_=omat_d.ap())
        sc = consts.tile([128, 1], f32)
        nc.sync.dma_start(out=sc[:], in_=scale_d.ap())
        bi = consts.tile([128, 1], f32)
        nc.sync.dma_start(out=bi[:], in_=bias_d.ap())
        qn = consts.tile([128, 1], f32)
        nc.sync.dma_start(out=qn[:], in_=qneg_d.ap())

        pot = potpool.tile([128, SUB], f32)

        loop_cm = (
            tc.For_i(0, bench_loops, 1, hint_engines=(mybir.EngineType.PE,))
            if bench_loops
            else None
        )
        if loop_cm is not None:
            loop_cm.__enter__()

        ngroups = (npair + group - 1) // group
        for g in range(ngroups):
            p0 = g * group
            gsz = min(group, npair - p0)
            zb = zpool.tile([128, group * SUB], f32r, tag="zb")
            nc.sync.dma_start(
                out=zb[:, : gsz * SUB],
                in_=zstack_d.ap()[:, p0 * SUB : (p0 + gsz) * SUB],
            )
            sq = sqpool.tile([128, group * SUB], f32r, tag="sq")
            # square on GPSIMD (otherwise idle) to keep DVE free for the recip
            nc.gpsimd.tensor_mul(sq[:, : gsz * SUB], zb[:, : gsz * SUB], zb[:, : gsz * SUB])

            dt_ = dpool.tile([128, group * 2 * SUB], f32, tag="dt")
            rt = rpool.tile([128, group * 2 * SUB], f32r, tag="rt")

            for pp in range(gsz):
                d2 = pspool.tile([128, 2 * SUB], f32, tag="d2")
                # adjacent MMs on disjoint row-halves (base partitions 0 / 64)
                # with independent PSUM banks -> PE runs them concurrently
                for h in (0, 1):
                    nc.tensor.matmul(
                        d2[:, h * SUB : (h + 1) * SUB],
                        lhsT=kz[64 * h : 64 * h + 64, :],
                        rhs=zb[64 * h : 64 * h + 64, pp * SUB : (pp + 1) * SUB],
                        start=True,
                        stop=False,
                    )
                for h in (0, 1):
                    nc.tensor.matmul(
                        d2[:, h * SUB : (h + 1) * SUB],
                        lhsT=ks[64 * h : 64 * h + 64, :],
                        rhs=sq[64 * h : 64 * h + 64, pp * SUB : (pp + 1) * SUB],
                        start=False,
                        stop=True,
                    )
                # dtil = sqrt(scale_j * d2 + bias_j) = d_ij / mu_j  over the pair
                nc.scalar.activation(
                    out=dt_[:, pp * 2 * SUB : (pp + 1) * 2 * SUB],
                    in_=d2[:],
                    func=AF.Sqrt,
                    bias=bi[:],
                    scale=sc[:],
                )
            # r = mu_j / d_ij over the whole group
            c_ = RECIP_APPROX_FAST_CONSTS
            nc.vector._custom_dve(
                RECIPROCAL_APPROX_FAST,
                out=rt[:, : gsz * 2 * SUB],
                in0=dt_[:, : gsz * 2 * SUB],
                s0=c_["s0"],
                s1=c_["s1"],
                imm2=c_["imm2"],
            )
            for pp in range(gsz):
                pair = p0 + pp
                for h in (0, 1):
                    t = 2 * pair + h
                    nc.tensor.matmul(
                        pot[:],
                        lhsT=om[:, 128 - t : 256 - t],
                        rhs=rt[:, (pp * 2 + h) * SUB : (pp * 2 + h + 1) * SUB],
                        start=(t == 0),
                        stop=(t == nsub - 1),
                        skip_group_check=True,
                    )

        if loop_cm is not None:
            loop_cm.__exit__(None, None, None)

        # ---- postlude: e = exp(-|pot - q_pot| / TEMP), fused row sums ----
        ab = consts.tile([128, SUB], f32)
        nc.scalar.activation(out=ab[:], in_=pot[:], func=AF.Abs, bias=qn[:], scale=1.0)
        e = consts.tile([128, SUB], f32)
        s1 = consts.tile([128, 1], f32)
        nc.scalar.activation(
            out=e[:], in_=ab[:], func=AF.Exp, scale=-1.0 / TEMP, accum_out=s1[:]
        )
        nc.sync.dma_start(out=eout_d.ap(), in_=e[0:nsub, :])
        nc.sync.dma_start(out=ssum_d.ap(), in_=s1[:])

    nc.compile()
    return nc


def _host_query(query_z, centers, mus):
    """Replicates reference's query-side math in f32 numpy (C-sized, tiny)."""
    q = query_z.astype(np.float32)          # (1, D)
    c = centers.astype(np.float32)          # (C, D)
    m = mus.astype(np.float32)              # (C,)
    diff = c[None, :, :] - q[:, None, :]    # (1, C, D)
    r2 = np.maximum(np.sum(diff * diff, axis=-1, keepdims=True), np.float32(1e-12))
    r = np.sqrt(r2)
    q_out = q + np.float32(DT) * np.sum(
        m[None, :, None] * diff / (r**3 + np.float32(EPS)), axis=1
    )
    # q_pot = sum_j mu_j / (dist + eps)
    d = np.sqrt(
        np.maximum(
            np.sum(q * q, axis=-1, keepdims=True)
            + np.sum(c * c, axis=-1)[None, :]
            - 2.0 * (q @ c.T),
            np.float32(1e-12),
        )
    )
    q_pot = np.sum(m[None, :] / (d + np.float32(EPS)), axis=1)[0]
    return q_out.astype(np.float32), np.float32(q_pot)


def _prep_inputs(candidate_z, centers, mus, q_pot):
    """Build per-core input maps (layout prep only; all N-sized compute is on-device)."""
    z = np.ascontiguousarray(candidate_z, dtype=np.float32)
    zpad = np.zeros((NPAD, D), dtype=np.float32)
    zpad[:N] = z

    c = centers.astype(np.float32)
    m = mus.astype(np.float32)
    cT2 = (-2.0 * c.T).astype(np.float32)               # (D, C)
    kxmz = np.vstack([cT2, cT2]).astype(np.float32)     # (128, 128)
    kxms = np.ones((128, 128), dtype=np.float32)
    omat = np.zeros((128, 256), dtype=np.float32)
    omat[:, 128] = 1.0
    inv_mu2 = (1.0 / (m * m)).astype(np.float32)
    csq = np.sum(c * c, axis=1).astype(np.float32)
    scalev = inv_mu2.reshape(128, 1)
    biasv = (csq * inv_mu2).reshape(128, 1).astype(np.float32)
    qneg = np.full((128, 1), -q_pot, dtype=np.float32)

    in_maps = []
    for core in range(NCORES):
        zc = zpad[core * NPC : (core + 1) * NPC]                  # (63488, 64)
        v = zc.reshape(NPAIR, 2, SUB, D)                          # [pair, h, i, d]
        zstack = np.ascontiguousarray(
            v.transpose(1, 3, 0, 2).reshape(128, NPAIR * SUB)
        )
        in_maps.append(
            dict(
                zstack=zstack,
                kxmz=kxmz,
                kxms=kxms,
                omat=omat,
                scalev=scalev,
                biasv=biasv,
                qneg=qneg,
            )
        )
    return in_maps


# test.py can flip these for profiling
TRACE = False
LAST_RESULTS = None


def kernel(query_z, candidate_z, centers, mus):
    global _NC, LAST_RESULTS
    _ensure_path()
    from concourse.bass_utils import run_bass_kernel_spmd

    query_z = np.asarray(query_z, dtype=np.float32)
    candidate_z = np.asarray(candidate_z, dtype=np.float32)
    centers = np.asarray(centers, dtype=np.float32)
    mus = np.asarray(mus, dtype=np.float32)

    q_out, q_pot = _host_query(query_z, centers, mus)
    in_maps = _prep_inputs(candidate_z, centers, mus, q_pot)

    if _NC is None:
        _NC = _build_program()

    kw = {}
    if TRACE:
        kw = dict(trace=True)
    res = run_bass_kernel_spmd(_NC, in_maps, list(range(NCORES)), **kw)
    LAST_RESULTS = res

    e = np.concatenate([res.results[c]["eout"].reshape(-1) for c in range(NCORES)])
    ssum = np.float64(0.0)
    for c in range(NCORES):
        ssum += res.results[c]["ssum"].astype(np.float64).sum()
    pad_sum = e[N:].astype(np.float64).sum()
    gsum = np.float32(ssum - pad_sum)

    attention = (e[:N] / gsum).astype(np.float32)
    return q_out, attention


if __name__ == "__main__":
    # smoke test with random data (no reference comparison)
    rng = np.random.default_rng(0)
    out = kernel(
        rng.standard_normal((1, D), dtype=np.float32),
        rng.standard_normal((N, D), dtype=np.float32),
        rng.standard_normal((C, D), dtype=np.float32),
        (rng.random(C, dtype=np.float32) * 0.5 + 0.1),
    )
    print("query_output:", out[0].shape, "attention:", out[1].shape, out[1].sum())


# revision 12
# speedup vs baseline: 1.1153x; 1.1153x over previous
"""Trainium2 Bass kernel for nn_AttentionWeightedRetrieval.

Math (reference):
    d_ij   = ||z_i - c_j||                         (N x C distances)
    c_pot_i = sum_j mu_j / (d_ij + eps)            (N,)
    q_pot   = same for the single query row
    logits  = -|q_pot - c_pot| / TEMP              (N,)  -- always <= 0
    attention = softmax(logits)                    (N,)
    query_output = query + dt * sum_j mu_j (c_j - q) / (r^3 + eps)   (1, D)

Device strategy (per core, candidates row-sharded 8 ways):
    - Host pre-transposes candidates into a "stacked" layout
      zstack[64*h + d, pair*512 + i] = z[(2*pair + h)*512 + i, d]
      so each matmul contracts over D on the partition axis with zero
      on-device transposes and fully contiguous DMA.
    - PE: d2acc = |z_i|^2 - 2 z.c_j via two K=64 matmuls into PSUM
      (lhsT = -2*centers^T for the z part, all-ones for the z^2 part;
      z^2 computed by one DVE multiply over the stacked tile).
    - ACT: dtil = sqrt(scale_j * d2acc + bias_j) = d_ij / mu_j
      with scale_j = 1/mu_j^2, bias_j = |c_j|^2/mu_j^2 (per-partition APs).
    - DVE: r = reciprocal_approx_fast(dtil) = mu_j / d_ij  (~51 ULP).
    - PE: pot_t = ones-indicator matmul accumulates sum_j r into row t of a
      single persistent PSUM bank -> after 124 subtiles the bank holds all
      63488 per-core potentials.
    - ACT: e = exp(-|pot - q_pot|/TEMP) (+ fused per-partition row sums).
    - Global softmax denominator: logits <= 0 by construction so no max
      subtraction is needed; per-core partial sums are combined at gather
      time (distributed-softmax all-gather-sum-exp, done on host).
"""

import numpy as np

N = 500000
C = 128
D = 64
EPS = 1e-6
TEMP = 0.1
DT = 0.1

NCORES = 8
SUB = 512            # candidates per matmul subtile
NSUB = 124           # subtiles per core
NPAIR = NSUB // 2
NPC = NSUB * SUB     # 63488 candidates per core
NPAD = NCORES * NPC  # 507904 padded total
GROUP = 4            # pairs per DMA / DVE batch

_NC = None           # compiled program cache (program is input-independent)


def _ensure_path():
    try:
        import concourse  # noqa: F401
    except ImportError:
        import sys
        for p in ("/opt/trn_rl_repo", "/root/.axon_site/_ro/trn_rl_repo"):
            if p not in sys.path:
                sys.path.insert(0, p)


def _build_program(nsub=NSUB, group=GROUP, bench_loops=None):
    """Build + compile the per-core Bass program (identical on all cores).

    bench_loops: if set, wrap the main tile loop in a hardware For_i that
    repeats it that many times (timing harness only — output is garbage
    except for the final iteration's pot, which is actually still correct
    since each iteration restarts the pot accumulation).
    """
    _ensure_path()
    from contextlib import ExitStack

    import concourse.bass as bass  # noqa: F401
    import concourse.tile as tile
    from concourse import bacc, mybir
    from concourse.dve_ops import RECIP_APPROX_FAST_CONSTS, RECIPROCAL_APPROX_FAST

    f32 = mybir.dt.float32
    f32r = mybir.dt.float32r  # fp32 bits, fast reduced-precision matmul path
    AF = mybir.ActivationFunctionType
    npair = nsub // 2

    nc = bacc.Bacc("TRN2", target_bir_lowering=False, debug=False)

    zstack_d = nc.dram_tensor("zstack", [128, npair * SUB], f32r, kind="ExternalInput")
    kxmz_d = nc.dram_tensor("kxmz", [128, 128], f32r, kind="ExternalInput")
    kxms_d = nc.dram_tensor("kxms", [128, 128], f32r, kind="ExternalInput")
    omat_d = nc.dram_tensor("omat", [128, 256], f32r, kind="ExternalInput")
    scale_d = nc.dram_tensor("scalev", [128, 1], f32, kind="ExternalInput")
    bias_d = nc.dram_tensor("biasv", [128, 1], f32, kind="ExternalInput")
    qneg_d = nc.dram_tensor("qneg", [128, 1], f32, kind="ExternalInput")
    eout_d = nc.dram_tensor("eout", [nsub, SUB], f32, kind="ExternalOutput")
    ssum_d = nc.dram_tensor("ssum", [128, 1], f32, kind="ExternalOutput")

    with tile.TileContext(nc) as tc, ExitStack() as ctx:
        consts = ctx.enter_context(tc.tile_pool(name="consts", bufs=1))
        zpool = ctx.enter_context(tc.tile_pool(name="zp", bufs=3))
        sqpool = ctx.enter_context(tc.tile_pool(name="sqp", bufs=3))
        dpool = ctx.enter_context(tc.tile_pool(name="dp", bufs=3))
        rpool = ctx.enter_context(tc.tile_pool(name="rp", bufs=3))
        pspool = ctx.enter_context(tc.tile_pool(name="ps", bufs=3, space="PSUM"))
        potpool = ctx.enter_context(tc.tile_pool(name="potp", bufs=1, space="PSUM"))

        kz = consts.tile([128, 128], f32r)
        nc.sync.dma_start(out=kz[:], in_=kxmz_d.ap())
        ks = consts.tile([128, 128], f32r)
        nc.sync.dma_start(out=ks[:], in_=kxms_d.ap())
        om = consts.tile([128, 256], f32r)
        nc.sync.dma_start(out=om[:], in_=omat_d.ap())
        sc = consts.tile([128, 1], f32)
        nc.sync.dma_start(out=sc[:], in_=scale_d.ap())
        bi = consts.tile([128, 1], f32)
        nc.sync.dma_start(out=bi[:], in_=bias_d.ap())
        qn = consts.tile([128, 1], f32)
        nc.sync.dma_start(out=qn[:], in_=qneg_d.ap())

        pot = potpool.tile([128, SUB], f32)

        loop_cm = (
            tc.For_i(0, bench_loops, 1, hint_engines=(mybir.EngineType.PE,))
            if bench_loops
            else None
        )
        if loop_cm is not None:
            loop_cm.__enter__()

        ngroups = (npair + group - 1) // group
        for g in range(ngroups):
            p0 = g * group
            gsz = min(group, npair - p0)
            zb = zpool.tile([128, group * SUB], f32r, tag="zb")
            nc.sync.dma_start(
                out=zb[:, : gsz * SUB],
                in_=zstack_d.ap()[:, p0 * SUB : (p0 + gsz) * SUB],
            )
            sq = sqpool.tile([128, group * SUB], f32r, tag="sq")
            # square on GPSIMD (otherwise idle) to keep DVE free for the recip
            nc.gpsimd.tensor_mul(sq[:, : gsz * SUB], zb[:, : gsz * SUB], zb[:, : gsz * SUB])

            dt_ = dpool.tile([128, group * 2 * SUB], f32, tag="dt")
            rt = rpool.tile([128, group * 2 * SUB], f32r, tag="rt")

            for pp in range(gsz):
                d2 = pspool.tile([128, 2 * SUB], f32, tag="d2")
                # adjacent MMs on disjoint row-halves (base partitions 0 / 64)
                # with independent PSUM banks -> PE runs them concurrently
                for h in (0, 1):
                    nc.tensor.matmul(
                        d2[:, h * SUB : (h + 1) * SUB],
                        lhsT=kz[64 * h : 64 * h + 64, :],
                        rhs=zb[64 * h : 64 * h + 64, pp * SUB : (pp + 1) * SUB],
                        start=True,
                        stop=False,
                    )
                for h in (0, 1):
                    nc.tensor.matmul(
                        d2[:, h * SUB : (h + 1) * SUB],
                        lhsT=ks[64 * h : 64 * h + 64, :],
                        rhs=sq[64 * h : 64 * h + 64, pp * SUB : (pp + 1) * SUB],
                        start=False,
                        stop=True,
                    )
                # dtil = sqrt(scale_j * d2 + bias_j) = d_ij / mu_j  over the pair
                nc.scalar.activation(
                    out=dt_[:, pp * 2 * SUB : (pp + 1) * 2 * SUB],
                    in_=d2[:],
                    func=AF.Sqrt,
                    bias=bi[:],
                    scale=sc[:],
                )
            # r = mu_j / d_ij over the whole group
            c_ = RECIP_APPROX_FAST_CONSTS
            nc.vector._custom_dve(
                RECIPROCAL_APPROX_FAST,
                out=rt[:, : gsz * 2 * SUB],
                in0=dt_[:, : gsz * 2 * SUB],
                s0=c_["s0"],
                s1=c_["s1"],
                imm2=c_["imm2"],
            )
            for pp in range(gsz):
                pair = p0 + pp
                for h in (0, 1):
                    t = 2 * pair + h
                    nc.tensor.matmul(
                        pot[:],
                        lhsT=om[:, 128 - t : 256 - t],
                        rhs=rt[:, (pp * 2 + h) * SUB : (pp * 2 + h + 1) * SUB],
                        start=(t == 0),
                        stop=(t == nsub - 1),
                        skip_group_check=True,
                    )

        if loop_cm is not None:
            loop_cm.__exit__(None, None, None)

        # ---- postlude: e = exp(-|pot - q_pot| / TEMP), fused row sums ----
        ab = consts.tile([128, SUB], f32)
        nc.scalar.activation(out=ab[:], in_=pot[:], func=AF.Abs, bias=qn[:], scale=1.0)
        e = consts.tile([128, SUB], f32)
        s1 = consts.tile([128, 1], f32)
        nc.scalar.activation(
            out=e[:], in_=ab[:], func=AF.Exp, scale=-1.0 / TEMP, accum_out=s1[:]
        )
        nc.sync.dma_start(out=eout_d.ap(), in_=e[0:nsub, :])
        nc.sync.dma_start(out=ssum_d.ap(), in_=s1[:])

    nc.compile()
    return nc


def _host_query(query_z, centers, mus):
    """Replicates reference's query-side math in f32 numpy (C-sized, tiny)."""
    q = query_z.astype(np.float32)          # (1, D)
    c = centers.astype(np.float32)          # (C, D)
    m = mus.astype(np.float32)              # (C,)
    diff = c[None, :, :] - q[:, None, :]    # (1, C, D)
    r2 = np.maximum(np.sum(diff * diff, axis=-1, keepdims=True), np.float32(1e-12))
    r = np.sqrt(r2)
    q_out = q + np.float32(DT) * np.sum(
        m[None, :, None] * diff / (r**3 + np.float32(EPS)), axis=1
    )
    # q_pot = sum_j mu_j / (dist + eps)
    d = np.sqrt(
        np.maximum(
            np.sum(q * q, axis=-1, keepdims=True)
            + np.sum(c * c, axis=-1)[None, :]
            - 2.0 * (q @ c.T),
            np.float32(1e-12),
        )
    )
    q_pot = np.sum(m[None, :] / (d + np.float32(EPS)), axis=1)[0]
    return q_out.astype(np.float32), np.float32(q_pot)


def _prep_inputs(candidate_z, centers, mus, q_pot):
    """Build per-core input maps (layout prep only; all N-sized compute is on-device)."""
    z = np.ascontiguousarray(candidate_z, dtype=np.float32)
    zpad = np.zeros((NPAD, D), dtype=np.float32)
    zpad[:N] = z

    c = centers.astype(np.float32)
    m = mus.astype(np.float32)
    cT2 = (-2.0 * c.T).astype(np.float32)               # (D, C)
    kxmz = np.vstack([cT2, cT2]).astype(np.float32)     # (128, 128)
    kxms = np.ones((128, 128), dtype=np.float32)
    omat = np.zeros((128, 256), dtype=np.float32)
    omat[:, 128] = 1.0
    inv_mu2 = (1.0 / (m * m)).astype(np.float32)
    csq = np.sum(c * c, axis=1).astype(np.float32)
    scalev = inv_mu2.reshape(128, 1)
    biasv = (csq * inv_mu2).reshape(128, 1).astype(np.float32)
    qneg = np.full((128, 1), -q_pot, dtype=np.float32)

    in_maps = []
    for core in range(NCORES):
        zc = zpad[core * NPC : (core + 1) * NPC]                  # (63488, 64)
        v = zc.reshape(NPAIR, 2, SUB, D)                          # [pair, h, i, d]
        zstack = np.ascontiguousarray(
            v.transpose(1, 3, 0, 2).reshape(128, NPAIR * SUB)
        )
        in_maps.append(
            dict(
                zstack=zstack,
                kxmz=kxmz,
                kxms=kxms,
                omat=omat,
                scalev=scalev,
                biasv=biasv,
                qneg=qneg,
            )
        )
    return in_maps


# test.py can flip these for profiling
TRACE = False
LAST_RESULTS = None


def kernel(query_z, candidate_z, centers, mus):
    global _NC, LAST_RESULTS
    _ensure_path()
    from concourse.bass_utils import run_bass_kernel_spmd

    query_z = np.asarray(query_z, dtype=np.float32)
    candidate_z = np.asarray(candidate_z, dtype=np.float32)
    centers = np.asarray(centers, dtype=np.float32)
    mus = np.asarray(mus, dtype=np.float32)

    q_out, q_pot = _host_query(query_z, centers, mus)
    in_maps = _prep_inputs(candidate_z, centers, mus, q_pot)

    if _NC is None:
        _NC = _build_program()

    kw = {}
    if TRACE:
        kw = dict(trace=True)
    # one retry: a transient NRT_EXEC_UNIT_UNRECOVERABLE has been observed to
    # clear on the next attempt
    try:
        res = run_bass_kernel_spmd(_NC, in_maps, list(range(NCORES)), **kw)
    except Exception:
        import time as _time

        _time.sleep(5)
        res = run_bass_kernel_spmd(_NC, in_maps, list(range(NCORES)), **kw)
    LAST_RESULTS = res

    e = np.concatenate([res.results[c]["eout"].reshape(-1) for c in range(NCORES)])
    ssum = np.float64(0.0)
    for c in range(NCORES):
        ssum += res.results[c]["ssum"].astype(np.float64).sum()
    pad_sum = e[N:].astype(np.float64).sum()
    gsum = np.float32(ssum - pad_sum)

    attention = (e[:N] / gsum).astype(np.float32)
    return q_out, attention


if __name__ == "__main__":
    # smoke test with random data (no reference comparison)
    rng = np.random.default_rng(0)
    out = kernel(
        rng.standard_normal((1, D), dtype=np.float32),
        rng.standard_normal((N, D), dtype=np.float32),
        rng.standard_normal((C, D), dtype=np.float32),
        (rng.random(C, dtype=np.float32) * 0.5 + 0.1),
    )
    print("query_output:", out[0].shape, "attention:", out[1].shape, out[1].sum())
